# revision 6
# baseline (speedup 1.0000x reference)
"""Trainium2 Bass kernel for nn_DiffusionHead: 100-step diffusion sampling of a
tiny MLP head (130->128->128->1) over a batch of 262144 rows.

Strategy (pure data parallel over 8 NeuronCores, 32768 rows/core):
  - Layout A on chip: features d on the 128 partitions, batch n on the free dim.
  - context @ W1[:128] is recomputed each step by the TensorEngine directly in
    PSUM (it is otherwise underutilized); the x-dependent part is a rank-1
    matmul accumulated into the same PSUM bank.
  - The per-step time embedding term time_emb[t]*W1[129] + b1 is a per-partition
    vector folded into the ScalarEngine's Silu bias operand (bias table built on
    device once).
  - Layer 3 (h2 @ W3) runs as M=1 matmuls col-tiled into a corner of the z1
    PSUM tile; pred is drained via VectorE to SBUF and DMA-reshaped into a
    square [128,128] layout where the x-update runs on VectorE with
    compile-time schedule constants folded in as immediates.
  - Batch is split in two halves per core so one half's x-recurrence tail
    (drain + update + scatter) hides under the other half's compute.
"""

import os
import numpy as np
import ml_dtypes

import concourse.bass as bass
import concourse.bacc as bacc
import concourse.mybir as mybir
from concourse import tile
from concourse import bass_utils

B = 262144
D = 128
T_STEPS = 100
N_CORES = 8
NPC = B // N_CORES          # 32768 rows per core
HALF = NPC // 2             # 16384
QUADS = HALF // 1024        # 16 z1-tiles per half (1024 cols each)
BETA_START = 1e-4
BETA_END = 0.02

F32 = mybir.dt.float32
BF16 = mybir.dt.bfloat16


def _schedule(n_steps):
    """Compile-time diffusion schedule constants (pure linspace math; no input
    data involved). Computed in float64 for accuracy."""
    betas = np.linspace(BETA_START, BETA_END, T_STEPS, dtype=np.float64)
    alphas = 1.0 - betas
    acp = np.cumprod(alphas)
    a_t = 1.0 / np.sqrt(alphas)                       # x coefficient
    b_t = -betas / (np.sqrt(1.0 - acp) * np.sqrt(alphas))  # pred coefficient
    c_t = np.sqrt(betas)                              # eps coefficient
    return a_t, b_t, c_t


def build(n_steps=T_STEPS, dt=BF16, skip=()):
    nc = bacc.Bacc("TRN2", target_bir_lowering=False, debug=False)

    # ---------------- DRAM tensors (per-core inputs) ----------------
    ctxT = nc.dram_tensor("ctxT", [D, NPC], dt, kind="ExternalInput").ap()
    noise = nc.dram_tensor("noise", [T_STEPS, NPC], F32, kind="ExternalInput").ap()
    x0 = nc.dram_tensor("x0", [NPC], F32, kind="ExternalInput").ap()
    W1a_d = nc.dram_tensor("W1a", [D, D], dt, kind="ExternalInput").ap()
    w1x_d = nc.dram_tensor("w1x", [1, D], dt, kind="ExternalInput").ap()
    w1t_d = nc.dram_tensor("w1t", [1, D], F32, kind="ExternalInput").ap()
    W2_d = nc.dram_tensor("W2", [D, D], dt, kind="ExternalInput").ap()
    W3_d = nc.dram_tensor("W3", [D, 1], dt, kind="ExternalInput").ap()
    b1_d = nc.dram_tensor("b1", [D, 1], F32, kind="ExternalInput").ap()
    b2_d = nc.dram_tensor("b2", [D, 1], F32, kind="ExternalInput").ap()
    b3_d = nc.dram_tensor("b3", [1, 1], F32, kind="ExternalInput").ap()
    temb_d = nc.dram_tensor("temb", [1, T_STEPS], F32, kind="ExternalInput").ap()
    xout = nc.dram_tensor("xout", [NPC], F32, kind="ExternalOutput").ap()

    a_t, b_t, c_t = _schedule(n_steps)
    ts_list = list(range(T_STEPS - 1, T_STEPS - 1 - n_steps, -1))

    with tile.TileContext(nc) as tc:
        with (
            tc.tile_pool(name="const", bufs=1) as const_pool,
            tc.tile_pool(name="ctx", bufs=1) as ctx_pool,
            tc.tile_pool(name="h1", bufs=3) as h1_pool,
            tc.tile_pool(name="h2", bufs=3) as h2_pool,
            tc.tile_pool(name="pstage", bufs=3) as pstage_pool,
            tc.tile_pool(name="predsq", bufs=3) as predsq_pool,
            tc.tile_pool(name="eps", bufs=4) as eps_pool,
            tc.tile_pool(name="xsq", bufs=3) as xsq_pool,
            tc.tile_pool(name="xrow", bufs=1) as xrow_pool,
            tc.tile_pool(name="xcast", bufs=3) as xcast_pool,
            tc.tile_pool(name="scratch", bufs=4) as scratch_pool,
            tc.tile_pool(name="z1p", bufs=2, space="PSUM") as z1_pool,
            tc.tile_pool(name="z2", bufs=2, space="PSUM") as z2_pool,
        ):
            # ---------------- load constants ----------------
            W1a = const_pool.tile([D, D], dt)
            nc.sync.dma_start(W1a[:], W1a_d)
            w1x = const_pool.tile([1, D], dt)
            nc.sync.dma_start(w1x[:], w1x_d)
            W2 = const_pool.tile([D, D], dt)
            nc.sync.dma_start(W2[:], W2_d)
            W3 = const_pool.tile([D, 1], dt)
            nc.sync.dma_start(W3[:], W3_d)
            # padded layer-3 stationaries: col j holds W3, other col 0, so the
            # two chunk preds of a z1-tile land on adjacent PSUM partitions
            W3c = []
            for j in range(2):
                w = const_pool.tile([D, 2], dt, tag=f"w3c{j}")
                nc.vector.memset(w[:], 0.0)
                nc.vector.tensor_copy(w[:, j:j + 1], W3[:])
                W3c.append(w)
            b1s = const_pool.tile([D, 1], F32)
            nc.sync.dma_start(b1s[:], b1_d)
            b2s = const_pool.tile([D, 1], F32)
            nc.sync.dma_start(b2s[:], b2_d)
            b3s = const_pool.tile([1, 1], F32)
            nc.sync.dma_start(b3s[:], b3_d)
            w1t = const_pool.tile([1, D], F32)
            nc.sync.dma_start(w1t[:], w1t_d)
            temb = const_pool.tile([1, T_STEPS], F32)
            nc.sync.dma_start(temb[:], temb_d)

            # bias_all[d, t] = b1[d] + time_emb[t] * W1[129, d]
            bias_ps = z2_pool.tile([D, T_STEPS], F32, tag="z2t")
            nc.tensor.matmul(bias_ps[:], w1t[:], temb[:], start=True, stop=True)
            bias_all = const_pool.tile([D, T_STEPS], F32)
            nc.vector.tensor_scalar_add(bias_all[:], bias_ps[:], b1s[:])

            # b3 broadcast to all 128 partitions (for the x-update)
            ones_r = const_pool.tile([1, D], F32)
            nc.vector.memset(ones_r[:], 1.0)
            b3_ps = z2_pool.tile([D, 1], F32, tag="z2t")
            nc.tensor.matmul(b3_ps[:], ones_r[:], b3s[:], start=True, stop=True)
            b3_bc = const_pool.tile([D, 1], F32)
            nc.vector.tensor_copy(b3_bc[:], b3_ps[:])

            # ---------------- load context (resident all steps) ----------------
            # split into chunks so multiple DMA engines run in parallel
            ctx_sb = ctx_pool.tile([D, NPC], dt)
            NCH = 8
            for ci in range(NCH):
                sl = slice(ci * (NPC // NCH), (ci + 1) * (NPC // NCH))
                nc.sync.dma_start(ctx_sb[:, sl], ctxT[:, sl])

            # ---------------- initial x ----------------
            # x square layout per half: [128, 128]; local col n = p*128 + f.
            x_sq = [None, None]
            x_row = [None, None]
            for h in range(2):
                xs = xsq_pool.tile([D, HALF // D], F32, tag=f"xsq{h}")
                nc.sync.dma_start(
                    xs[:],
                    x0[h * HALF:(h + 1) * HALF].rearrange("(p f) -> p f", p=D),
                )
                x_sq[h] = xs
                xr = xrow_pool.tile([1, HALF], dt, tag=f"xrow{h}")
                if dt == F32:
                    nc.sync.dma_start(xr[:], xs[:])
                else:
                    xc = xcast_pool.tile([D, HALF // D], dt, tag=f"xcast{h}")
                    nc.vector.tensor_copy(xc[:], xs[:])
                    nc.sync.dma_start(xr[:], xc[:])
                x_row[h] = xr

            # ---------------- main loop ----------------
            # Software-pipelined over "slots" (one slot = 1024 batch cols):
            #   slot s: z1-MMs(q_s) + silu1 (FD=1024)        [front]
            #           L2 + one merged silu2 (FD=1024) q_{s-1}  [mid]
            #           L3 + pred drain for q_{s-2}          [back]
            # ACT stream alternates silu1(s), silu2(s-1): the z2 single-buffer
            # ping-pong (L2(s) after silu2(s-1)) hides under silu1(s+1).
            # PSUM: z1 2x2 banks + z2 1x2 banks + pred 2x1 banks = 8.
            from collections import deque

            half_info = {}

            def emit_front(si, t, h, q):
                hoff = h * HALF
                if q == 0:
                    eps = None
                    if t > 0:
                        eps = eps_pool.tile([D, HALF // D], F32)
                        # jax.lax.scan pairs ts[i]=T-1-i with noise[i]
                        nc.sync.dma_start(
                            eps[:],
                            noise[si, hoff:hoff + HALF].rearrange(
                                "(p f) -> p f", p=D),
                        )
                    pred_sq = predsq_pool.tile([D, HALF // D], F32)
                    half_info[(si, h)] = (eps, pred_sq)
                co = hoff + q * 1024
                lo = q * 1024
                zp = z1_pool.tile([D, 1024], F32)
                for k in range(2):
                    nc.tensor.matmul(zp[:, 512 * k:512 * (k + 1)], W1a[:],
                                     ctx_sb[:, co + 512 * k:co + 512 * (k + 1)],
                                     start=True, stop=False)
                for k in range(2):
                    nc.tensor.matmul(zp[:, 512 * k:512 * (k + 1)], w1x[:],
                                     x_row[h][:, lo + 512 * k:lo + 512 * (k + 1)],
                                     start=False, stop=True)
                h1 = h1_pool.tile([D, 1024], dt)
                nc.scalar.activation(
                    h1[:], zp[:, 0:1024],
                    mybir.ActivationFunctionType.Silu,
                    bias=bias_all[:, t:t + 1], scale=1.0,
                )
                return {"si": si, "t": t, "h": h, "q": q, "h1": h1, "zp": zp}

            def emit_mid(rec):
                z2t = z2_pool.tile([D, 1024], F32, tag="z2t")
                for k in range(2):
                    nc.tensor.matmul(
                        z2t[:, 512 * k:512 * (k + 1)], W2[:],
                        rec["h1"][:, 512 * k:512 * (k + 1)],
                        start=True, stop=True)
                h2 = h2_pool.tile([D, 1024], dt)
                nc.scalar.activation(
                    h2[:], z2t[:, 0:1024],
                    mybir.ActivationFunctionType.Silu,
                    bias=b2s[:], scale=1.0,
                )
                rec["h2"] = h2

            def emit_back(rec):
                si, t, h, q = rec["si"], rec["t"], rec["h"], rec["q"]
                eps, pred_sq = half_info[(si, h)]
                # L3 pred lands in a corner of this record's (already-consumed)
                # z1 PSUM tile: no extra PSUM banks needed.
                pp = rec["zp"]
                for j in range(2):
                    nc.tensor.matmul(pp[0:2, 0:512], W3c[j][:],
                                     rec["h2"][:, 512 * j:512 * (j + 1)],
                                     start=(j == 0), stop=(j == 1))
                ps = pstage_pool.tile([2, 512], F32)
                nc.vector.tensor_copy(ps[:], pp[0:2, 0:512])
                nc.sync.dma_start(pred_sq[8 * q:8 * q + 8, :], ps[:])
                if q == HALF // 1024 - 1:
                    emit_x_update(si, t, h, eps, pred_sq)
            def emit_x_update(si, t, h, eps, pred_sq):
                at = float(a_t[t])
                bt = float(b_t[t])
                ct = float(c_t[t])
                last = si == n_steps - 1
                hoff = h * HALF
                u = scratch_pool.tile([D, HALF // D], F32, tag="xu")
                nc.vector.tensor_scalar_mul(u[:], x_sq[h][:], at)
                p = scratch_pool.tile([D, HALF // D], F32, tag="xp")
                nc.vector.tensor_scalar(
                    p[:], pred_sq[:], b3_bc[:], bt,
                    mybir.AluOpType.add, mybir.AluOpType.mult,
                )
                xs_new = xsq_pool.tile([D, HALF // D], F32, tag=f"xsq{h}")
                if t > 0:
                    v = scratch_pool.tile([D, HALF // D], F32, tag="xv")
                    nc.vector.tensor_tensor(v[:], u[:], p[:],
                                            mybir.AluOpType.add)
                    e = scratch_pool.tile([D, HALF // D], F32, tag="xe")
                    nc.vector.tensor_scalar_mul(e[:], eps[:], ct)
                    nc.vector.tensor_tensor(xs_new[:], v[:], e[:],
                                            mybir.AluOpType.add)
                else:
                    nc.vector.tensor_tensor(xs_new[:], u[:], p[:],
                                            mybir.AluOpType.add)
                x_sq[h] = xs_new
                if last:
                    nc.sync.dma_start(
                        xout[hoff:hoff + HALF].rearrange("(p f) -> p f", p=D),
                        xs_new[:],
                    )
                else:
                    xr = xrow_pool.tile([1, HALF], dt, tag=f"xrow{h}")
                    if dt == F32:
                        nc.sync.dma_start(xr[:], xs_new[:])
                    else:
                        xc = xcast_pool.tile([D, HALF // D], dt,
                                             tag=f"xcast{h}")
                        nc.vector.tensor_copy(xc[:], xs_new[:])
                        nc.sync.dma_start(xr[:], xc[:])
                    x_row[h] = xr

            # Per-slot emit order is back(s-2), mid(s-1), front(s) so the PE
            # stream is L3(s-2), L2(s-1), z1-MMs(s): front(s) reuses the z1
            # buffer of s-2, whose corner L3(s-2) writes, so L3 must be issued
            # first (engines execute strictly in order).
            pipe = deque()
            for si, t in enumerate(ts_list):
                for h in range(2):
                    for q in range(HALF // 1024):
                        if len(pipe) >= 2:
                            emit_back(pipe.popleft())
                        if pipe:
                            emit_mid(pipe[-1])
                        pipe.append(emit_front(si, t, h, q))
            emit_mid(pipe[-1])
            while pipe:
                emit_back(pipe.popleft())

    nc.compile()
    return nc


_BUILD_CACHE = {}


def _get_nc(n_steps, dt):
    key = (n_steps, str(dt))
    if key not in _BUILD_CACHE:
        _BUILD_CACHE[key] = build(n_steps, dt)
    return _BUILD_CACHE[key]


def _prep_in_maps(context, x_init, noise, W1, b1, W2, b2, W3, b3, time_emb, dt):
    np_dt = np.float32 if dt == F32 else ml_dtypes.bfloat16
    in_maps = []
    W1a = np.ascontiguousarray(W1[:D].astype(np_dt))
    w1x = np.ascontiguousarray(W1[D:D + 1].astype(np_dt))
    w1t = np.ascontiguousarray(W1[D + 1:D + 2].astype(np.float32))
    W2c = np.ascontiguousarray(W2.astype(np_dt))
    W3c = np.ascontiguousarray(W3.astype(np_dt))
    b1c = np.ascontiguousarray(b1.reshape(D, 1).astype(np.float32))
    b2c = np.ascontiguousarray(b2.reshape(D, 1).astype(np.float32))
    b3c = np.ascontiguousarray(b3.reshape(1, 1).astype(np.float32))
    tec = np.ascontiguousarray(time_emb.reshape(1, T_STEPS).astype(np.float32))
    for c in range(N_CORES):
        s = slice(c * NPC, (c + 1) * NPC)
        in_maps.append({
            "ctxT": np.ascontiguousarray(context[s].T.astype(np_dt)),
            "noise": np.ascontiguousarray(noise[:, s, 0].astype(np.float32)),
            "x0": np.ascontiguousarray(x_init[s, 0].astype(np.float32)),
            "W1a": W1a, "w1x": w1x, "w1t": w1t,
            "W2": W2c, "W3": W3c,
            "b1": b1c, "b2": b2c, "b3": b3c,
            "temb": tec,
        })
    return in_maps


def run(inputs, n_steps=T_STEPS, dt=None, trace=False, tmpdir=None):
    if dt is None:
        dt = F32 if os.environ.get("K_DT", "bf16") == "f32" else BF16
    nc = _get_nc(n_steps, dt)
    in_maps = _prep_in_maps(**{k: np.asarray(v) for k, v in inputs.items()}, dt=dt)
    res = bass_utils.run_bass_kernel_spmd(
        nc, in_maps, list(range(N_CORES)), trace=trace, tmpdir=tmpdir,
    )
    out = np.concatenate([res.results[c]["xout"] for c in range(N_CORES)])
    return out.reshape(B, 1).astype(np.float32), res


def kernel(**inputs):
    out, _ = run(inputs)
    return out

